# revision 15
# baseline (speedup 1.0000x reference)
"""AdaptiveGN-Patches-Hadamard kernel for 8 TRN2 NeuronCores.

Reference computation (per sample b):
  - split (128, 256, 256) image into 4x4 patches of 64x64
  - per-patch GroupNorm over 32 groups (4 channels x 64 x 64 each), affine w/b
  - out = xn * (1 + silu(y)) elementwise, same spatial layout

Sharding: pure data parallel, one batch sample per core (batch=8, cores=8).
Layout on core: channels (128) on partitions, spatial on the free dim.

x is loaded as full-width bands (64 KiB contiguous per partition -> top DMA
efficiency) with an f32->bf16 cast on the SWDGE path; y is loaded / the
output stored as f32 half-bands on the two HWDGE rings.  Stats are per
channel x patch (S on DVE reduce + ACT Copy-accum, Q on ACT Square-accum),
then combined across each group's 4 channels with two tiny TensorEngine
matmuls against constant group matrices.
"""

import os
import sys

sys.path.insert(0, "/opt/trn_rl_repo")

from contextlib import ExitStack

import numpy as np

import concourse.bacc as bacc
import concourse.bass as bass
import concourse.mybir as mybir
import concourse.tile as tile
from concourse.bass_utils import run_bass_kernel_spmd

C = 128  # channels
H = 256
W = 256
NP = 4  # patches per side
P = 64  # patch size
G = 32  # groups
CG = C // G  # channels per group
EPS = 1e-5
FP = mybir.dt.float32
BF = mybir.dt.bfloat16

BAND_H = P  # 64 rows per band
HALF_W = 128  # half of the width
PATCH_N = P * P * CG  # elements per group-patch (16384)


def _build_graph() -> bass.Bass:
    nc = bacc.Bacc(
        "TRN2",
        target_bir_lowering=False,
        debug=False,
        num_devices=8,
    )

    x_d = nc.declare_dram_parameter("x", [C, H, W], FP, isOutput=False)
    y_d = nc.declare_dram_parameter("y", [C, H, W], FP, isOutput=False)
    w_d = nc.declare_dram_parameter("wvec", [C, 1], FP, isOutput=False)
    b_d = nc.declare_dram_parameter("bvec", [C, 1], FP, isOutput=False)
    g_d = nc.declare_dram_parameter("gmat", [C, G], FP, isOutput=False)
    m_d = nc.declare_dram_parameter("bmat", [G, C], FP, isOutput=False)
    out_d = nc.declare_dram_parameter("out", [C, H, W], FP, isOutput=True)

    with tile.TileContext(nc) as tc, ExitStack() as ctx:
        singles = ctx.enter_context(tc.tile_pool(name="singles", bufs=1))
        xpool = ctx.enter_context(tc.tile_pool(name="xp", bufs=2))
        ypool = ctx.enter_context(tc.tile_pool(name="yp", bufs=3))
        scrp = ctx.enter_context(tc.tile_pool(name="scr", bufs=1))
        statp = ctx.enter_context(tc.tile_pool(name="stats", bufs=4))
        smallp = ctx.enter_context(tc.tile_pool(name="small", bufs=4))
        ps_g = ctx.enter_context(tc.tile_pool(name="psg", bufs=2, space="PSUM"))
        ps_c = ctx.enter_context(tc.tile_pool(name="psc", bufs=2, space="PSUM"))

        g_sb = singles.tile([C, G], FP)
        nc.sync.dma_start(out=g_sb, in_=g_d[:, :])
        m_sb = singles.tile([G, C], FP)
        nc.sync.dma_start(out=m_sb, in_=m_d[:, :])
        w_sb = singles.tile([C, 1], FP)
        nc.sync.dma_start(out=w_sb, in_=w_d[:, :])
        b_sb = singles.tile([C, 1], FP)
        nc.sync.dma_start(out=b_sb, in_=b_d[:, :])
        eps_sb = singles.tile([G, 1], FP)
        nc.vector.memset(eps_sb, EPS)

        for i in range(NP):  # band of rows
            # full-width band of x, f32->bf16 cast on the SWDGE (gpsimd) ring
            xt = xpool.tile([C, BAND_H, W], BF)
            nc.gpsimd.dma_start(
                out=xt,
                in_=x_d[:, i * BAND_H : (i + 1) * BAND_H, :],
            )

            # per-channel, per-patch S = sum(x), Q = sum(x^2), 4 patches
            st = statp.tile([C, 8], FP)  # [j, (S, Q)] interleaved
            stv = st[:].rearrange("p (a b) -> p a b", b=2)
            sq_scr = scrp.tile([C, BAND_H, P], BF)
            for j in range(NP):
                xpatch = xt[:, :, j * P : (j + 1) * P]
                if j % 2 == 0:
                    nc.vector.reduce_sum(
                        out=stv[:, j, 0:1],
                        in_=xpatch,
                        axis=mybir.AxisListType.XY,
                    )
                else:
                    nc.scalar.activation(
                        out=sq_scr,
                        in_=xpatch,
                        func=mybir.ActivationFunctionType.Copy,
                        accum_out=stv[:, j, 0:1],
                    )
                nc.scalar.activation(
                    out=sq_scr,
                    in_=xpatch,
                    func=mybir.ActivationFunctionType.Square,
                    accum_out=stv[:, j, 1:2],
                )

            # group-combine: pg[g, (j,(mean,e2))] = (1/N) * group sum
            pg = ps_g.tile([G, 8], FP)
            nc.tensor.matmul(pg, g_sb, st[:], start=True, stop=True)

            gs = statp.tile([G, 8], FP)
            nc.vector.tensor_copy(gs, pg)
            gsv = gs[:].rearrange("p (a b) -> p a b", b=2)
            # var_g = e2_g - mean_g^2 ; invstd = 1/sqrt(var_g + eps)
            sqg = smallp.tile([G, 4], FP)
            nc.vector.tensor_mul(sqg, gsv[:, :, 0], gsv[:, :, 0])
            nc.vector.tensor_sub(gsv[:, :, 1], gsv[:, :, 1], sqg)
            # std to a separate tile (ACT), reciprocal back into gs (DVE)
            # so gs stays written by a single engine for the next matmul
            std_t = smallp.tile([G, 4], FP)
            nc.scalar.activation(
                out=std_t,
                in_=gsv[:, :, 1],
                func=mybir.ActivationFunctionType.Sqrt,
                bias=eps_sb[:],
                scale=1.0,
            )
            nc.vector.reciprocal(gsv[:, :, 1], std_t)

            # broadcast group stats back to channels
            pc = ps_c.tile([C, 8], FP)
            nc.tensor.matmul(pc, m_sb, gs[:], start=True, stop=True)
            pcv = pc[:].rearrange("p (a b) -> p a b", b=2)

            # A = invstd * weight ; B = bias - mean * A  (per chan, patch)
            ab = statp.tile([C, 8], FP)
            abv = ab[:].rearrange("p (a b) -> p a b", b=2)
            nc.vector.tensor_scalar_mul(abv[:, :, 0], pcv[:, :, 1], w_sb[:])
            tm = smallp.tile([C, 4], FP)
            nc.vector.tensor_mul(tm, pcv[:, :, 0], abv[:, :, 0])
            nc.vector.tensor_scalar(
                out=abv[:, :, 1],
                in0=tm,
                scalar1=b_sb[:],
                scalar2=-1.0,
                op0=mybir.AluOpType.subtract,
                op1=mybir.AluOpType.mult,
            )

            # xn = x * A + B, in place, per patch (DVE tensor_scalar, bf16 4x)
            for j in range(NP):
                nc.vector.tensor_scalar(
                    out=xt[:, :, j * P : (j + 1) * P],
                    in0=xt[:, :, j * P : (j + 1) * P],
                    scalar1=abv[:, j, 0:1],
                    scalar2=abv[:, j, 1:2],
                    op0=mybir.AluOpType.mult,
                    op1=mybir.AluOpType.add,
                )

            for hh in range(2):  # half of the columns: y path + gate + store
                yt = ypool.tile([C, BAND_H, HALF_W], FP)
                nc.sync.dma_start(
                    out=yt,
                    in_=y_d[:, i * BAND_H : (i + 1) * BAND_H,
                            hh * HALF_W : (hh + 1) * HALF_W],
                )
                yflat = yt[:].rearrange("p a b -> p (a b)")
                nc.scalar.activation(
                    out=yflat, in_=yflat,
                    func=mybir.ActivationFunctionType.Silu,
                )
                # out = (silu(y) + 1) * xn  (fused on DVE)
                nc.vector.scalar_tensor_tensor(
                    out=yflat,
                    in0=yflat,
                    scalar=1.0,
                    in1=xt[:, :, hh * HALF_W : (hh + 1) * HALF_W],
                    op0=mybir.AluOpType.add,
                    op1=mybir.AluOpType.mult,
                )
                # store on the ACT HWDGE ring so it can't head-of-line
                # block the y loads on the sync ring
                nc.scalar.dma_start(
                    out=out_d[:, i * BAND_H : (i + 1) * BAND_H,
                              hh * HALF_W : (hh + 1) * HALF_W],
                    in_=yt,
                )

    nc.compile()
    return nc


_GRAPH_CACHE: bass.Bass | None = None


def _get_graph() -> bass.Bass:
    global _GRAPH_CACHE
    if _GRAPH_CACHE is None:
        _GRAPH_CACHE = _build_graph()
    return _GRAPH_CACHE


def kernel(x: np.ndarray, y: np.ndarray, weight: np.ndarray, bias: np.ndarray,
           **_unused) -> np.ndarray:
    assert x.shape == (8, C, H, W) and y.shape == (8, C, H, W)
    n_cores = 8

    gmat = np.zeros((C, G), np.float32)
    gmat[np.arange(C), np.arange(C) // CG] = 1.0 / PATCH_N
    bmat = np.zeros((G, C), np.float32)
    bmat[np.arange(C) // CG, np.arange(C)] = 1.0

    wvec = np.ascontiguousarray(weight.astype(np.float32).reshape(C, 1))
    bvec = np.ascontiguousarray(bias.astype(np.float32).reshape(C, 1))

    in_maps = [
        {
            "x": np.ascontiguousarray(x[i], dtype=np.float32),
            "y": np.ascontiguousarray(y[i], dtype=np.float32),
            "wvec": wvec,
            "bvec": bvec,
            "gmat": gmat,
            "bmat": bmat,
        }
        for i in range(n_cores)
    ]

    nc = _get_graph()
    trace = bool(int(os.environ.get("KERNEL_TRACE", "0")))
    res = run_bass_kernel_spmd(
        nc, in_maps, core_ids=list(range(n_cores)), trace=trace,
    )
    if trace and res.exec_time_ns is not None:
        print(f"HW exec time: {res.exec_time_ns} ns")

    out = np.stack([np.asarray(res.results[i]["out"]) for i in range(n_cores)])
    return out.astype(np.float32)


# revision 16
# speedup vs baseline: 1.0902x; 1.0902x over previous
"""AdaptiveGN-Patches-Hadamard kernel for 8 TRN2 NeuronCores.

Reference computation (per sample b):
  - split (128, 256, 256) image into 4x4 patches of 64x64
  - per-patch GroupNorm over 32 groups (4 channels x 64 x 64 each), affine w/b
  - out = xn * (1 + silu(y)) elementwise, same spatial layout

Sharding: pure data parallel, one batch sample per core (batch=8, cores=8).
Layout on core: channels (128) on partitions, spatial on the free dim.

x is loaded as full-width bands (64 KiB contiguous per partition -> top DMA
efficiency) with an f32->bf16 cast on the SWDGE path; y is loaded / the
output stored as f32 half-bands on the two HWDGE rings.  Stats are per
channel x patch (S on DVE reduce + ACT Copy-accum, Q on ACT Square-accum),
then combined across each group's 4 channels with two tiny TensorEngine
matmuls against constant group matrices.
"""

import os
import sys

sys.path.insert(0, "/opt/trn_rl_repo")

from contextlib import ExitStack

import numpy as np

import concourse.bacc as bacc
import concourse.bass as bass
import concourse.mybir as mybir
import concourse.tile as tile
from concourse.bass_utils import run_bass_kernel_spmd

C = 128  # channels
H = 256
W = 256
NP = 4  # patches per side
P = 64  # patch size
G = 32  # groups
CG = C // G  # channels per group
EPS = 1e-5
FP = mybir.dt.float32
BF = mybir.dt.bfloat16

BAND_H = P  # 64 rows per band
HALF_W = 128  # half of the width
PATCH_N = P * P * CG  # elements per group-patch (16384)


def _build_graph() -> bass.Bass:
    nc = bacc.Bacc(
        "TRN2",
        target_bir_lowering=False,
        debug=False,
        num_devices=8,
    )

    x_d = nc.declare_dram_parameter("x", [C, H, W], FP, isOutput=False)
    y_d = nc.declare_dram_parameter("y", [C, H, W], FP, isOutput=False)
    w_d = nc.declare_dram_parameter("wvec", [C, 1], FP, isOutput=False)
    b_d = nc.declare_dram_parameter("bvec", [C, 1], FP, isOutput=False)
    g_d = nc.declare_dram_parameter("gmat", [C, G], FP, isOutput=False)
    m_d = nc.declare_dram_parameter("bmat", [G, C], FP, isOutput=False)
    out_d = nc.declare_dram_parameter("out", [C, H, W], FP, isOutput=True)

    with tile.TileContext(nc) as tc, ExitStack() as ctx:
        singles = ctx.enter_context(tc.tile_pool(name="singles", bufs=1))
        xpool = ctx.enter_context(tc.tile_pool(name="xp", bufs=3))
        ypool = ctx.enter_context(tc.tile_pool(name="yp", bufs=2))
        scrp = ctx.enter_context(tc.tile_pool(name="scr", bufs=1))
        statp = ctx.enter_context(tc.tile_pool(name="stats", bufs=4))
        smallp = ctx.enter_context(tc.tile_pool(name="small", bufs=4))
        ps_g = ctx.enter_context(tc.tile_pool(name="psg", bufs=2, space="PSUM"))
        ps_c = ctx.enter_context(tc.tile_pool(name="psc", bufs=2, space="PSUM"))

        g_sb = singles.tile([C, G], FP)
        nc.sync.dma_start(out=g_sb, in_=g_d[:, :])
        m_sb = singles.tile([G, C], FP)
        nc.sync.dma_start(out=m_sb, in_=m_d[:, :])
        w_sb = singles.tile([C, 1], FP)
        nc.sync.dma_start(out=w_sb, in_=w_d[:, :])
        b_sb = singles.tile([C, 1], FP)
        nc.sync.dma_start(out=b_sb, in_=b_d[:, :])
        eps_sb = singles.tile([G, 1], FP)
        nc.vector.memset(eps_sb, EPS)

        for i in range(NP):  # band of rows
            # full-width band of x, f32->bf16 cast on the SWDGE (gpsimd) ring
            xt = xpool.tile([C, BAND_H, W], BF)
            nc.gpsimd.dma_start(
                out=xt,
                in_=x_d[:, i * BAND_H : (i + 1) * BAND_H, :],
            )

            # per-channel, per-patch S = sum(x), Q = sum(x^2), 4 patches
            st = statp.tile([C, 8], FP)  # [j, (S, Q)] interleaved
            stv = st[:].rearrange("p (a b) -> p a b", b=2)
            sq_scr = scrp.tile([C, BAND_H, P], BF)
            for j in range(NP):
                xpatch = xt[:, :, j * P : (j + 1) * P]
                if j % 2 == 0:
                    nc.vector.reduce_sum(
                        out=stv[:, j, 0:1],
                        in_=xpatch,
                        axis=mybir.AxisListType.XY,
                    )
                else:
                    nc.scalar.activation(
                        out=sq_scr,
                        in_=xpatch,
                        func=mybir.ActivationFunctionType.Copy,
                        accum_out=stv[:, j, 0:1],
                    )
                nc.scalar.activation(
                    out=sq_scr,
                    in_=xpatch,
                    func=mybir.ActivationFunctionType.Square,
                    accum_out=stv[:, j, 1:2],
                )

            # group-combine: pg[g, (j,(mean,e2))] = (1/N) * group sum
            pg = ps_g.tile([G, 8], FP)
            nc.tensor.matmul(pg, g_sb, st[:], start=True, stop=True)

            gs = statp.tile([G, 8], FP)
            nc.vector.tensor_copy(gs, pg)
            gsv = gs[:].rearrange("p (a b) -> p a b", b=2)
            # var_g = e2_g - mean_g^2 ; invstd = 1/sqrt(var_g + eps)
            sqg = smallp.tile([G, 4], FP)
            nc.vector.tensor_mul(sqg, gsv[:, :, 0], gsv[:, :, 0])
            nc.vector.tensor_sub(gsv[:, :, 1], gsv[:, :, 1], sqg)
            # std to a separate tile (ACT), reciprocal back into gs (DVE)
            # so gs stays written by a single engine for the next matmul
            std_t = smallp.tile([G, 4], FP)
            nc.scalar.activation(
                out=std_t,
                in_=gsv[:, :, 1],
                func=mybir.ActivationFunctionType.Sqrt,
                bias=eps_sb[:],
                scale=1.0,
            )
            nc.vector.reciprocal(gsv[:, :, 1], std_t)

            # broadcast group stats back to channels
            pc = ps_c.tile([C, 8], FP)
            nc.tensor.matmul(pc, m_sb, gs[:], start=True, stop=True)
            pcv = pc[:].rearrange("p (a b) -> p a b", b=2)

            # A = invstd * weight ; B = bias - mean * A  (per chan, patch)
            ab = statp.tile([C, 8], FP)
            abv = ab[:].rearrange("p (a b) -> p a b", b=2)
            nc.vector.tensor_scalar_mul(abv[:, :, 0], pcv[:, :, 1], w_sb[:])
            tm = smallp.tile([C, 4], FP)
            nc.vector.tensor_mul(tm, pcv[:, :, 0], abv[:, :, 0])
            nc.vector.tensor_scalar(
                out=abv[:, :, 1],
                in0=tm,
                scalar1=b_sb[:],
                scalar2=-1.0,
                op0=mybir.AluOpType.subtract,
                op1=mybir.AluOpType.mult,
            )

            # xn = x * A + B, in place, per patch (DVE tensor_scalar, bf16 4x)
            for j in range(NP):
                nc.vector.tensor_scalar(
                    out=xt[:, :, j * P : (j + 1) * P],
                    in0=xt[:, :, j * P : (j + 1) * P],
                    scalar1=abv[:, j, 0:1],
                    scalar2=abv[:, j, 1:2],
                    op0=mybir.AluOpType.mult,
                    op1=mybir.AluOpType.add,
                )

            for hh in range(2):  # half of the columns: y path + gate + store
                yt = ypool.tile([C, BAND_H, HALF_W], FP)
                nc.sync.dma_start(
                    out=yt,
                    in_=y_d[:, i * BAND_H : (i + 1) * BAND_H,
                            hh * HALF_W : (hh + 1) * HALF_W],
                )
                yflat = yt[:].rearrange("p a b -> p (a b)")
                nc.scalar.activation(
                    out=yflat, in_=yflat,
                    func=mybir.ActivationFunctionType.Silu,
                )
                # out = (silu(y) + 1) * xn  (fused on DVE)
                nc.vector.scalar_tensor_tensor(
                    out=yflat,
                    in0=yflat,
                    scalar=1.0,
                    in1=xt[:, :, hh * HALF_W : (hh + 1) * HALF_W],
                    op0=mybir.AluOpType.add,
                    op1=mybir.AluOpType.mult,
                )
                # store on the ACT HWDGE ring so it can't head-of-line
                # block the y loads on the sync ring
                nc.scalar.dma_start(
                    out=out_d[:, i * BAND_H : (i + 1) * BAND_H,
                              hh * HALF_W : (hh + 1) * HALF_W],
                    in_=yt,
                )

    nc.compile()
    return nc


_GRAPH_CACHE: bass.Bass | None = None


def _get_graph() -> bass.Bass:
    global _GRAPH_CACHE
    if _GRAPH_CACHE is None:
        _GRAPH_CACHE = _build_graph()
    return _GRAPH_CACHE


def kernel(x: np.ndarray, y: np.ndarray, weight: np.ndarray, bias: np.ndarray,
           **_unused) -> np.ndarray:
    assert x.shape == (8, C, H, W) and y.shape == (8, C, H, W)
    n_cores = 8

    gmat = np.zeros((C, G), np.float32)
    gmat[np.arange(C), np.arange(C) // CG] = 1.0 / PATCH_N
    bmat = np.zeros((G, C), np.float32)
    bmat[np.arange(C) // CG, np.arange(C)] = 1.0

    wvec = np.ascontiguousarray(weight.astype(np.float32).reshape(C, 1))
    bvec = np.ascontiguousarray(bias.astype(np.float32).reshape(C, 1))

    in_maps = [
        {
            "x": np.ascontiguousarray(x[i], dtype=np.float32),
            "y": np.ascontiguousarray(y[i], dtype=np.float32),
            "wvec": wvec,
            "bvec": bvec,
            "gmat": gmat,
            "bmat": bmat,
        }
        for i in range(n_cores)
    ]

    nc = _get_graph()
    trace = bool(int(os.environ.get("KERNEL_TRACE", "0")))
    res = run_bass_kernel_spmd(
        nc, in_maps, core_ids=list(range(n_cores)), trace=trace,
    )
    if trace and res.exec_time_ns is not None:
        print(f"HW exec time: {res.exec_time_ns} ns")

    out = np.stack([np.asarray(res.results[i]["out"]) for i in range(n_cores)])
    return out.astype(np.float32)


# revision 21
# speedup vs baseline: 1.1438x; 1.0492x over previous
"""AdaptiveGN-Patches-Hadamard kernel for 8 TRN2 NeuronCores.

Reference computation (per sample b):
  - split (128, 256, 256) image into 4x4 patches of 64x64
  - per-patch GroupNorm over 32 groups (4 channels x 64 x 64 each), affine w/b
  - out = xn * (1 + silu(y)) elementwise, same spatial layout

Sharding: pure data parallel, one batch sample per core (batch=8, cores=8).
Layout on core: channels (128) on partitions, spatial on the free dim.
Per-channel per-patch sums S (DVE reduce) and sums of squares Q (ACT Square
with accum_out) are combined across the 4 channels of each group with two
tiny TensorEngine matmuls against constant group matrices.
"""

import os
import sys

sys.path.insert(0, "/opt/trn_rl_repo")

from contextlib import ExitStack

import numpy as np

import concourse.bacc as bacc
import concourse.bass as bass
import concourse.mybir as mybir
import concourse.tile as tile
from concourse.bass_utils import run_bass_kernel_spmd

C = 128  # channels
H = 256
W = 256
NP = 4  # patches per side
P = 64  # patch size
G = 32  # groups
CG = C // G  # channels per group
EPS = 1e-5
FP = mybir.dt.float32
BF = mybir.dt.bfloat16

BAND_H = P  # 64 rows per band
HALF_W = 128  # half of the width -> 2 patches per half-band
PATCH_N = P * P * CG  # elements per group-patch (16384)


def _build_graph() -> bass.Bass:
    nc = bacc.Bacc(
        "TRN2",
        target_bir_lowering=False,
        debug=False,
        num_devices=8,
    )

    x_d = nc.declare_dram_parameter("x", [C, H, W], FP, isOutput=False)
    y_d = nc.declare_dram_parameter("y", [C, H, W], FP, isOutput=False)
    w_d = nc.declare_dram_parameter("wvec", [C, 1], FP, isOutput=False)
    b_d = nc.declare_dram_parameter("bvec", [C, 1], FP, isOutput=False)
    g_d = nc.declare_dram_parameter("gmat", [C, G], FP, isOutput=False)
    m_d = nc.declare_dram_parameter("bmat", [G, C], FP, isOutput=False)
    out_d = nc.declare_dram_parameter("out", [C, H, W], FP, isOutput=True)

    with tile.TileContext(nc) as tc, ExitStack() as ctx:
        singles = ctx.enter_context(tc.tile_pool(name="singles", bufs=1))
        xpool = ctx.enter_context(tc.tile_pool(name="xp", bufs=4))
        ypool = ctx.enter_context(tc.tile_pool(name="yp", bufs=3))
        scrp = ctx.enter_context(tc.tile_pool(name="scr", bufs=1))
        statp = ctx.enter_context(tc.tile_pool(name="stats", bufs=6))
        smallp = ctx.enter_context(tc.tile_pool(name="small", bufs=6))
        ps_g = ctx.enter_context(tc.tile_pool(name="psg", bufs=4, space="PSUM"))
        ps_c = ctx.enter_context(tc.tile_pool(name="psc", bufs=4, space="PSUM"))

        g_sb = singles.tile([C, G], FP)
        nc.sync.dma_start(out=g_sb, in_=g_d[:, :])
        m_sb = singles.tile([G, C], FP)
        nc.sync.dma_start(out=m_sb, in_=m_d[:, :])
        w_sb = singles.tile([C, 1], FP)
        nc.sync.dma_start(out=w_sb, in_=w_d[:, :])
        b_sb = singles.tile([C, 1], FP)
        nc.sync.dma_start(out=b_sb, in_=b_d[:, :])
        eps_sb = singles.tile([G, 1], FP)
        nc.vector.memset(eps_sb, EPS)

        for i in range(NP):  # band of rows
            for hh in range(2):  # half of the columns
                # f32->bf16 cast on the SWDGE (gpsimd) ring, off the sync ring
                xt = xpool.tile([C, BAND_H, HALF_W], BF)
                nc.gpsimd.dma_start(
                    out=xt,
                    in_=x_d[:, i * BAND_H : (i + 1) * BAND_H,
                            hh * HALF_W : (hh + 1) * HALF_W],
                )
                yt = ypool.tile([C, BAND_H, HALF_W], FP)
                nc.sync.dma_start(
                    out=yt,
                    in_=y_d[:, i * BAND_H : (i + 1) * BAND_H,
                            hh * HALF_W : (hh + 1) * HALF_W],
                )

                # per-channel, per-patch S = sum(x), Q = sum(x^2).
                # Q on ACT (Square + accum_out, out tile is waste); S split
                # between DVE (reduce) and ACT (Copy + accum_out) for balance.
                st = statp.tile([C, 4], FP)  # [j2, (S, Q)] interleaved
                stv = st[:].rearrange("p (a b) -> p a b", b=2)
                sq_scr = scrp.tile([C, BAND_H, P], BF)
                for j2 in range(2):  # patch within the half-band
                    xpatch = xt[:, :, j2 * P : (j2 + 1) * P]
                    if j2 == 0:
                        nc.vector.reduce_sum(
                            out=stv[:, j2, 0:1],
                            in_=xpatch,
                            axis=mybir.AxisListType.XY,
                        )
                    else:
                        nc.scalar.activation(
                            out=sq_scr,
                            in_=xpatch,
                            func=mybir.ActivationFunctionType.Copy,
                            accum_out=stv[:, j2, 0:1],
                        )
                    nc.scalar.activation(
                        out=sq_scr,
                        in_=xpatch,
                        func=mybir.ActivationFunctionType.Square,
                        accum_out=stv[:, j2, 1:2],
                    )

                # group-combine: pg[g, (j2,(mean,e2))] = (1/N) * group sum
                pg = ps_g.tile([G, 4], FP)
                nc.tensor.matmul(pg, g_sb, st[:], start=True, stop=True)

                gs = statp.tile([G, 4], FP)
                nc.vector.tensor_copy(gs, pg)
                gsv = gs[:].rearrange("p (a b) -> p a b", b=2)
                # var_g = e2_g - mean_g^2 ; invstd = 1/sqrt(var_g + eps)
                sqg = smallp.tile([G, 2], FP)
                nc.vector.tensor_mul(sqg, gsv[:, :, 0], gsv[:, :, 0])
                nc.vector.tensor_sub(gsv[:, :, 1], gsv[:, :, 1], sqg)
                # std to a separate tile (ACT), reciprocal back into gs (DVE)
                # so gs stays written by a single engine for the next matmul
                std_t = smallp.tile([G, 2], FP)
                nc.scalar.activation(
                    out=std_t,
                    in_=gsv[:, :, 1],
                    func=mybir.ActivationFunctionType.Sqrt,
                    bias=eps_sb[:],
                    scale=1.0,
                )
                nc.vector.reciprocal(gsv[:, :, 1], std_t)

                # broadcast group stats back to channels
                pc = ps_c.tile([C, 4], FP)
                nc.tensor.matmul(pc, m_sb, gs[:], start=True, stop=True)
                pcv = pc[:].rearrange("p (a b) -> p a b", b=2)

                # A = invstd * weight ; B = bias - mean * A  (per chan, patch)
                ab = statp.tile([C, 4], FP)
                abv = ab[:].rearrange("p (a b) -> p a b", b=2)
                nc.vector.tensor_scalar_mul(abv[:, :, 0], pcv[:, :, 1], w_sb[:])
                tm = smallp.tile([C, 2], FP)
                nc.vector.tensor_mul(tm, pcv[:, :, 0], abv[:, :, 0])
                nc.vector.tensor_scalar(
                    out=abv[:, :, 1],
                    in0=tm,
                    scalar1=b_sb[:],
                    scalar2=-1.0,
                    op0=mybir.AluOpType.subtract,
                    op1=mybir.AluOpType.mult,
                )

                # xn = x * A + B, in place, per patch (DVE tensor_scalar)
                for j2 in range(2):
                    nc.vector.tensor_scalar(
                        out=xt[:, :, j2 * P : (j2 + 1) * P],
                        in0=xt[:, :, j2 * P : (j2 + 1) * P],
                        scalar1=abv[:, j2, 0:1],
                        scalar2=abv[:, j2, 1:2],
                        op0=mybir.AluOpType.mult,
                        op1=mybir.AluOpType.add,
                    )

                # gate: out = (silu(y) + 1) * xn; silu on ACT, fused
                # scalar_tensor_tensor on DVE
                yflat = yt[:].rearrange("p a b -> p (a b)")
                xflat = xt[:].rearrange("p a b -> p (a b)")
                nc.scalar.activation(
                    out=yflat, in_=yflat,
                    func=mybir.ActivationFunctionType.Silu,
                )
                nc.vector.scalar_tensor_tensor(
                    out=yflat,
                    in0=yflat,
                    scalar=1.0,
                    in1=xflat,
                    op0=mybir.AluOpType.add,
                    op1=mybir.AluOpType.mult,
                )

                # store on the ACT HWDGE ring so it can't head-of-line
                # block the next iteration's loads on the sync ring
                nc.scalar.dma_start(
                    out=out_d[:, i * BAND_H : (i + 1) * BAND_H,
                              hh * HALF_W : (hh + 1) * HALF_W],
                    in_=yt,
                )

    nc.compile()
    return nc


_GRAPH_CACHE: bass.Bass | None = None


def _get_graph() -> bass.Bass:
    global _GRAPH_CACHE
    if _GRAPH_CACHE is None:
        _GRAPH_CACHE = _build_graph()
    return _GRAPH_CACHE


def kernel(x: np.ndarray, y: np.ndarray, weight: np.ndarray, bias: np.ndarray,
           **_unused) -> np.ndarray:
    assert x.shape == (8, C, H, W) and y.shape == (8, C, H, W)
    n_cores = 8

    gmat = np.zeros((C, G), np.float32)
    gmat[np.arange(C), np.arange(C) // CG] = 1.0 / PATCH_N
    bmat = np.zeros((G, C), np.float32)
    bmat[np.arange(C) // CG, np.arange(C)] = 1.0

    wvec = np.ascontiguousarray(weight.astype(np.float32).reshape(C, 1))
    bvec = np.ascontiguousarray(bias.astype(np.float32).reshape(C, 1))

    in_maps = [
        {
            "x": np.ascontiguousarray(x[i], dtype=np.float32),
            "y": np.ascontiguousarray(y[i], dtype=np.float32),
            "wvec": wvec,
            "bvec": bvec,
            "gmat": gmat,
            "bmat": bmat,
        }
        for i in range(n_cores)
    ]

    nc = _get_graph()
    trace = bool(int(os.environ.get("KERNEL_TRACE", "0")))
    res = run_bass_kernel_spmd(
        nc, in_maps, core_ids=list(range(n_cores)), trace=trace,
    )
    if trace and res.exec_time_ns is not None:
        print(f"HW exec time: {res.exec_time_ns} ns")

    out = np.stack([np.asarray(res.results[i]["out"]) for i in range(n_cores)])
    return out.astype(np.float32)
